# revision 8
# baseline (speedup 1.0000x reference)
"""Trainium2 Bass/Tile kernel for a 6-layer dual-stream encoder.

Strategy: data-parallel over batch (B=8) across the 8 NeuronCores; each core
runs the full 6-layer encoder for one batch element. Activations are kept
feature-major (D on partitions, S on the free dim) so weights serve as matmul
lhsT in their natural layout. LayerNorm statistics are computed with
ones-vector matmuls (partition-dim reduction). Attention computes both
Q@K^T (softmax + attention-map output) and K@Q^T (transposed scores for the
attn@K / attn@V contractions). Q/K/V and the transposed exp-scores are held
in bf16 to fit SBUF; residual streams and everything else are fp32.
"""

import sys

if "/opt/trn_rl_repo" not in sys.path:
    sys.path.insert(0, "/opt/trn_rl_repo")

import numpy as np

import concourse.bass as bass
import concourse.bacc as bacc
import concourse.tile as tile
from concourse import mybir
from concourse.bass_utils import run_bass_kernel_spmd

F32 = mybir.dt.float32
BF16 = mybir.dt.bfloat16
AF = mybir.ActivationFunctionType

L, D, H, DK, DF = 6, 1024, 16, 64, 4096
B, S = 8, 512
HD = H * DK            # 1024
KD = D // 128          # 8 feature tiles
SP = S // 128          # 4 token tiles
NPAIR = HD // 128      # 8 head pairs
NBLK = 4               # DF blocks of 1024
EPS = 1e-5
N_CORES = 8

# column-pack layout (within cols[l], shape (128, NCOL)):
C_BQ, C_BK, C_BXO, C_BMO = 0, 8, 16, 24
C_BX1, C_BM1 = 32, 64
C_BX2, C_BM2 = 96, 104
C_LN1X, C_LN1M = 112, 128      # g at +0, b at +8
C_LN2X, C_LN2M = 144, 160
C_LN3X, C_LN3M = 176, 192
NCOL = 208


def _build(n_layers=L):
    nc = bacc.Bacc("TRN2", target_bir_lowering=False, debug=False,
                   enable_asserts=False, num_devices=N_CORES)

    x0 = nc.dram_tensor("x0", [S, D], F32, kind="ExternalInput").ap()
    m0 = nc.dram_tensor("m0", [S, D], F32, kind="ExternalInput").ap()
    wq = nc.dram_tensor("Wq", [n_layers, D, HD], F32, kind="ExternalInput").ap()
    wk = nc.dram_tensor("Wk", [n_layers, D, HD], F32, kind="ExternalInput").ap()
    wv = nc.dram_tensor("Wv", [n_layers, D, HD], F32, kind="ExternalInput").ap()
    wxo = nc.dram_tensor("Wxo", [n_layers, HD, D], F32, kind="ExternalInput").ap()
    wmo = nc.dram_tensor("Wmo", [n_layers, HD, D], F32, kind="ExternalInput").ap()
    wx1 = nc.dram_tensor("Wx1", [n_layers, D, DF], F32, kind="ExternalInput").ap()
    wx2 = nc.dram_tensor("Wx2", [n_layers, DF, D], F32, kind="ExternalInput").ap()
    wm1 = nc.dram_tensor("Wm1", [n_layers, D, DF], F32, kind="ExternalInput").ap()
    wm2 = nc.dram_tensor("Wm2", [n_layers, DF, D], F32, kind="ExternalInput").ap()
    cols = nc.dram_tensor("cols", [n_layers, 128, NCOL], F32, kind="ExternalInput").ap()
    rows = nc.dram_tensor("rows", [n_layers, 2, HD], F32, kind="ExternalInput").ap()
    ion = nc.dram_tensor("ionc", [128, 640], F32, kind="ExternalInput").ap()

    sx_out = nc.dram_tensor("sx_out", [S, D], F32, kind="ExternalOutput").ap()
    sm_out = nc.dram_tensor("sm_out", [S, D], F32, kind="ExternalOutput").ap()
    attn_out = nc.dram_tensor("attn_out", [n_layers, H, S, S], F32,
                              kind="ExternalOutput").ap()

    with tile.TileContext(nc) as tc:
        with tc.tile_pool(name="sb", bufs=1) as sb, \
             tc.tile_pool(name="ps", bufs=8, space="PSUM") as psp:

            ion_t = sb.tile([128, 640], F32, tag="ion")
            nc.sync.dma_start(out=ion_t, in_=ion)
            ident = ion_t[:, 0:128]
            ones_col = ion_t[:, 128:129]     # (128,1) fp32 ones
            ones_row = ion_t[0:1, 128:256]   # (1,128) fp32 ones
            onesb = sb.tile([128, 16], BF16, tag="ionb")
            nc.vector.memset(onesb, 1.0)
            ones_col_b = onesb[:, 0:1]       # (128,1) bf16 ones
            eps_t = sb.tile([128, 1], F32, tag="epsc")
            nc.vector.memset(eps_t, EPS)

            def ln(src3, cbase, dst3):
                """LayerNorm over the partition (feature) dim.
                src3/dst3: [128, KD, S] fp32; cbase: colpack base (g at cbase,
                b at cbase+KD)."""
                ps1 = psp.tile([1, S], F32, tag="ps")
                ps2 = psp.tile([1, S], F32, tag="ps")
                for d in range(KD):
                    sq = sb.tile([128, S], F32, tag="sq", bufs=3)
                    nc.scalar.activation(sq, src3[:, d, :], AF.Square)
                    nc.tensor.matmul(ps1, ones_col, src3[:, d, :],
                                     start=(d == 0), stop=(d == KD - 1))
                    nc.tensor.matmul(ps2, ones_col, sq,
                                     start=(d == 0), stop=(d == KD - 1))
                vr = sb.tile([1, 4, S], F32, tag="vrow", bufs=1)
                mu, var, rstd, murstd = (vr[:, 0, :], vr[:, 1, :],
                                         vr[:, 2, :], vr[:, 3, :])
                nc.scalar.activation(mu, ps1, AF.Copy, scale=1.0 / D)
                nc.scalar.activation(var, ps2, AF.Copy, scale=1.0 / D)
                nc.vector.tensor_mul(rstd, mu, mu)           # mu^2 (scratch)
                nc.vector.tensor_sub(var, var, rstd)         # var
                nc.scalar.activation(var, var, AF.Sqrt,
                                     bias=eps_t[0:1, :])  # std
                nc.vector.reciprocal(rstd, var)              # 1/std
                nc.vector.tensor_mul(murstd, mu, rstd)       # mu/std
                psr = psp.tile([128, S], F32, tag="ps")
                psm = psp.tile([128, S], F32, tag="ps")
                nc.tensor.matmul(psr, ones_row, rstd, start=True, stop=True)
                nc.tensor.matmul(psm, ones_row, murstd, start=True, stop=True)
                for d in range(KD):
                    t = sb.tile([128, S], F32, tag="sq", bufs=3)
                    nc.vector.tensor_mul(t, src3[:, d, :], psr)
                    nc.vector.tensor_sub(t, t, psm)
                    nc.vector.tensor_scalar(
                        dst3[:, d, :], t,
                        colp[:, cbase + d:cbase + d + 1],
                        colp[:, cbase + KD + d:cbase + KD + d + 1],
                        op0=mybir.AluOpType.mult, op1=mybir.AluOpType.add)

            def wquarter(wsrc, l, q, colsl=None):
                """Load a (256, N<=1024) row-quarter of a weight matrix as
                [128, 2, N] (contraction rows on partitions)."""
                if colsl is None:
                    src = wsrc[l][q * 256:(q + 1) * 256, :]
                else:
                    src = wsrc[l][q * 256:(q + 1) * 256, colsl[0]:colsl[1]]
                n = src.shape[1]
                t = sb.tile([128, 2, n], F32, tag="wslab", bufs=3)
                nc.sync.dma_start(
                    out=t, in_=src.rearrange("(kd p) n -> p kd n", p=128))
                return t

            # ---------- input transposes: token-major -> feature-major ----
            sxv = sb.tile([128, KD, S], F32, tag="s5")
            smv = sb.tile([128, KD, S], F32, tag="s6")
            for (src, dst3) in ((x0, sxv), (m0, smv)):
                for i in range(SP):
                    stg = sb.tile([128, D], F32, tag="attn_sb", bufs=2)
                    nc.sync.dma_start(out=stg, in_=src[i * 128:(i + 1) * 128, :])
                    for d in range(KD):
                        pt = psp.tile([128, 128], F32, tag="ps")
                        nc.tensor.transpose(pt, stg[:, d * 128:(d + 1) * 128],
                                            ident)
                        nc.vector.tensor_copy(
                            dst3[:, d, i * 128:(i + 1) * 128], pt)

            for l in range(n_layers):
                colp = sb.tile([128, NCOL], F32, tag="colp", bufs=2)
                nc.sync.dma_start(out=colp, in_=cols[l])
                bkbc = sb.tile([128, 2, HD], F32, tag="bkbc")
                for r in range(2):
                    rsrc = rows[l, r]
                    bsrc = bass.AP(tensor=rsrc.tensor, offset=rsrc.offset,
                                   ap=[[0, 128]] + list(rsrc.ap))
                    nc.gpsimd.dma_start(out=bkbc[:, r, :], in_=bsrc)

                # ---------- A: LayerNorm 1 on both streams ----------
                xt3 = sb.tile([128, KD, S], F32, tag="s1")
                mt3 = sb.tile([128, KD, S], F32, tag="s2")
                ln(sxv, C_LN1X, xt3)
                ln(smv, C_LN1M, mt3)

                # ---------- B: projections ----------
                qkt = sb.tile([128, NPAIR, 1024], BF16, tag="s3")
                kv = sb.tile([128, SP, 2048], BF16, tag="s4")

                # KT: feature-major K^T -> qkt[:, pair, 512:1024]
                pls = [psp.tile([128, S], F32, tag="ps", name=f"pls{_i}")
                       for _i in range(8)]
                for q in range(4):
                    wt = wquarter(wk, l, q)
                    for k2 in range(2):
                        kd = q * 2 + k2
                        for m in range(NPAIR):
                            nc.tensor.matmul(
                                pls[m], wt[:, k2, m * 128:(m + 1) * 128],
                                xt3[:, kd, :],
                                start=(kd == 0), stop=(kd == KD - 1))
                for m in range(NPAIR):
                    nc.scalar.activation(qkt[:, m, 512:1024], pls[m], AF.Identity,
                                         bias=colp[:, C_BK + m:C_BK + m + 1])

                # K: token-major -> kv[:, j, 0:1024]
                pls = [psp.tile([128, S], F32, tag="ps", name=f"pls{_i}")
                       for _i in range(8)]
                for q in range(4):
                    wt = wquarter(wk, l, q)
                    for k2 in range(2):
                        kd = q * 2 + k2
                        for j in range(SP):
                            for n in range(2):
                                nc.tensor.matmul(
                                    pls[j * 2 + n],
                                    xt3[:, kd, j * 128:(j + 1) * 128],
                                    wt[:, k2, n * 512:(n + 1) * 512],
                                    start=(kd == 0), stop=(kd == KD - 1))
                for j in range(SP):
                    for n in range(2):
                        nc.vector.tensor_add(
                            kv[:, j, n * 512:(n + 1) * 512], pls[j * 2 + n],
                            bkbc[:, 0, n * 512:(n + 1) * 512])

                # V: token-major -> kv[:, j, 1024:2048]
                pls = [psp.tile([128, S], F32, tag="ps", name=f"pls{_i}")
                       for _i in range(8)]
                for q in range(4):
                    wt = wquarter(wv, l, q)
                    for k2 in range(2):
                        kd = q * 2 + k2
                        for j in range(SP):
                            for n in range(2):
                                nc.tensor.matmul(
                                    pls[j * 2 + n],
                                    mt3[:, kd, j * 128:(j + 1) * 128],
                                    wt[:, k2, n * 512:(n + 1) * 512],
                                    start=(kd == 0), stop=(kd == KD - 1))
                for j in range(SP):
                    for n in range(2):
                        nc.vector.tensor_add(
                            kv[:, j, 1024 + n * 512:1024 + (n + 1) * 512],
                            pls[j * 2 + n],
                            bkbc[:, 1, n * 512:(n + 1) * 512])

                # QT: feature-major Q^T -> qkt[:, pair, 0:512]
                pls = [psp.tile([128, S], F32, tag="ps", name=f"pls{_i}")
                       for _i in range(8)]
                for q in range(4):
                    wt = wquarter(wq, l, q)
                    for k2 in range(2):
                        kd = q * 2 + k2
                        for m in range(NPAIR):
                            nc.tensor.matmul(
                                pls[m], wt[:, k2, m * 128:(m + 1) * 128],
                                xt3[:, kd, :],
                                start=(kd == 0), stop=(kd == KD - 1))
                for m in range(NPAIR):
                    nc.scalar.activation(qkt[:, m, 0:512], pls[m], AF.Identity,
                                         bias=colp[:, C_BQ + m:C_BQ + m + 1])

                # ---------- C: attention ----------
                kbt = sb.tile([128, NPAIR, S], F32, tag="s7")
                vbt = sb.tile([128, NPAIR, S], F32, tag="s8")
                for h in range(H):
                    hd, po = h // 2, (h % 2) * 64
                    att = sb.tile([128, SP, S], F32, tag="attn_sb", bufs=2)
                    rec = sb.tile([128, 520], F32, tag="rec", bufs=2)
                    for i in range(SP):
                        pd = psp.tile([128, S], F32, tag="ps")
                        nc.tensor.matmul(
                            pd, qkt[po:po + 64, hd, i * 128:(i + 1) * 128],
                            qkt[po:po + 64, hd, 512:1024],
                            start=True, stop=True)
                        nc.scalar.activation(
                            att[:, i, :], pd, AF.Exp, scale=0.125,
                            accum_out=rec[:, 512 + i:513 + i])
                    nc.vector.reciprocal(rec[:, 516:520], rec[:, 512:516])
                    for i in range(SP):
                        nc.vector.tensor_scalar_mul(
                            att[:, i, :], att[:, i, :],
                            rec[:, 516 + i:517 + i])
                    nc.sync.dma_start(
                        out=attn_out[l, h].rearrange("(i p) j -> p i j", p=128),
                        in_=att)
                    # transposed scores
                    ext = sb.tile([128, SP, S], BF16, tag="expT", bufs=1)
                    for j in range(SP):
                        pdt = psp.tile([128, S], F32, tag="ps")
                        nc.tensor.matmul(
                            pdt,
                            qkt[po:po + 64, hd, 512 + j * 128:512 + (j + 1) * 128],
                            qkt[po:po + 64, hd, 0:512],
                            start=True, stop=True)
                        nc.scalar.activation(ext[:, j, :], pdt, AF.Exp,
                                             scale=0.125)
                    pss = psp.tile([1, S], F32, tag="ps")
                    for j in range(SP):
                        nc.tensor.matmul(pss, ones_col_b, ext[:, j, :],
                                         start=(j == 0), stop=(j == SP - 1))
                    rrow = sb.tile([1, S], F32, tag="rrow", bufs=1)
                    nc.vector.reciprocal(rrow, pss)
                    prb = psp.tile([128, S], F32, tag="ps")
                    nc.tensor.matmul(prb, ones_row, rrow, start=True, stop=True)
                    nc.vector.tensor_copy(rec[:, 0:512], prb)
                    pkb = psp.tile([64, S], F32, tag="ps")
                    pvb = psp.tile([64, S], F32, tag="ps")
                    for j in range(SP):
                        nc.tensor.matmul(pkb, kv[:, j, h * 64:(h + 1) * 64],
                                         ext[:, j, :],
                                         start=(j == 0), stop=(j == SP - 1))
                        nc.tensor.matmul(
                            pvb, kv[:, j, 1024 + h * 64:1024 + (h + 1) * 64],
                            ext[:, j, :],
                            start=(j == 0), stop=(j == SP - 1))
                    nc.vector.tensor_mul(kbt[po:po + 64, hd, :], pkb,
                                         rec[0:64, 0:512])
                    nc.vector.tensor_mul(vbt[po:po + 64, hd, :], pvb,
                                         rec[0:64, 0:512])

                # ---------- D: output projections + residual + LN2 --------
                xat = sb.tile([128, KD, S], F32, tag="s3")
                mat = sb.tile([128, KD, S], F32, tag="s4")
                for (wsrc, rhs3, dst3, cb) in ((wxo, kbt, xat, C_BXO),
                                               (wmo, vbt, mat, C_BMO)):
                    pls = [psp.tile([128, S], F32, tag="ps", name=f"pls{_i}")
                       for _i in range(8)]
                    for q in range(4):
                        wt = wquarter(wsrc, l, q)
                        for k2 in range(2):
                            kh = q * 2 + k2
                            for m in range(KD):
                                nc.tensor.matmul(
                                    pls[m], wt[:, k2, m * 128:(m + 1) * 128],
                                    rhs3[:, kh, :],
                                    start=(kh == 0), stop=(kh == NPAIR - 1))
                    for m in range(KD):
                        nc.scalar.activation(dst3[:, m, :], pls[m], AF.Identity,
                                             bias=colp[:, cb + m:cb + m + 1])

                s2px = sb.tile([128, KD, S], F32, tag="s7")
                for d in range(KD):
                    nc.vector.tensor_add(s2px[:, d, :], sxv[:, d, :],
                                         xat[:, d, :])
                sx2 = sb.tile([128, KD, S], F32, tag="s1")
                ln(s2px, C_LN2X, sx2)
                s2pm = sb.tile([128, KD, S], F32, tag="s8")
                for d in range(KD):
                    nc.vector.tensor_add(s2pm[:, d, :], smv[:, d, :],
                                         mat[:, d, :])
                sm2 = sb.tile([128, KD, S], F32, tag="s2")
                ln(s2pm, C_LN2M, sm2)

                # ---------- E: FFN on both streams ----------
                for (src3, w1, w2, cb1, cb2, res3, cln, dstv) in (
                        (xat, wx1, wx2, C_BX1, C_BX2, sx2, C_LN3X, sxv),
                        (mat, wm1, wm2, C_BM1, C_BM2, sm2, C_LN3M, smv)):
                    fxa = sb.tile([128, KD, S], F32, tag="s8")
                    for blk in range(NBLK):
                        hxb = sb.tile([128, 8, S], F32, tag="s7")
                        pls = [psp.tile([128, S], F32, tag="ps", name=f"plsf{_i}")
                               for _i in range(8)]
                        for q in range(4):
                            wt = wquarter(w1, l, q,
                                          colsl=(blk * 1024, (blk + 1) * 1024))
                            for k2 in range(2):
                                kd = q * 2 + k2
                                for df in range(8):
                                    nc.tensor.matmul(
                                        pls[df],
                                        wt[:, k2, df * 128:(df + 1) * 128],
                                        src3[:, kd, :],
                                        start=(kd == 0), stop=(kd == KD - 1))
                        for df in range(8):
                            c = cb1 + blk * 8 + df
                            nc.scalar.activation(hxb[:, df, :], pls[df],
                                                 AF.Relu,
                                                 bias=colp[:, c:c + 1])
                        pls2 = [psp.tile([128, S], F32, tag="ps", name=f"pls2{_i}")
                                for _i in range(8)]
                        for q2 in range(4):
                            src = w2[l][blk * 1024 + q2 * 256:
                                        blk * 1024 + (q2 + 1) * 256, :]
                            wt2 = sb.tile([128, 2, D], F32, tag="wslab",
                                          bufs=3)
                            nc.sync.dma_start(
                                out=wt2,
                                in_=src.rearrange("(df p) n -> p df n", p=128))
                            for d2 in range(2):
                                df = q2 * 2 + d2
                                for m in range(KD):
                                    nc.tensor.matmul(
                                        pls2[m],
                                        wt2[:, d2, m * 128:(m + 1) * 128],
                                        hxb[:, df, :],
                                        start=(df == 0), stop=(df == 7))
                        for m in range(KD):
                            if blk == 0:
                                nc.scalar.activation(
                                    fxa[:, m, :], pls2[m], AF.Identity,
                                    bias=colp[:, cb2 + m:cb2 + m + 1])
                            else:
                                nc.vector.tensor_add(fxa[:, m, :],
                                                     fxa[:, m, :], pls2[m])
                    for m in range(KD):
                        nc.vector.tensor_add(fxa[:, m, :], fxa[:, m, :],
                                             res3[:, m, :])
                    ln(fxa, cln, dstv)

            # ---------- output transposes ----------
            for (src3, dst) in ((sxv, sx_out), (smv, sm_out)):
                for i in range(SP):
                    stg = sb.tile([128, D], F32, tag="attn_sb", bufs=2)
                    for d in range(KD):
                        pt = psp.tile([128, 128], F32, tag="ps")
                        nc.tensor.transpose(
                            pt, src3[:, d, i * 128:(i + 1) * 128], ident)
                        nc.vector.tensor_copy(stg[:, d * 128:(d + 1) * 128], pt)
                    nc.sync.dma_start(out=dst[i * 128:(i + 1) * 128, :],
                                      in_=stg)

    nc.compile()
    return nc


_NC_CACHE = {}


def _get_nc(n_layers=L):
    if n_layers not in _NC_CACHE:
        _NC_CACHE[n_layers] = _build(n_layers)
    return _NC_CACHE[n_layers]


def _colform(v):
    """(nl, n) per-feature vectors -> (nl, 128, n//128) column form."""
    nl, n = v.shape
    return np.ascontiguousarray(
        v.reshape(nl, n // 128, 128).transpose(0, 2, 1))


def _pack_params(p, n_layers):
    cols = np.zeros((n_layers, 128, NCOL), np.float32)
    cols[:, :, C_BQ:C_BQ + 8] = _colform(p["bq"])
    cols[:, :, C_BK:C_BK + 8] = _colform(p["bk"])
    cols[:, :, C_BXO:C_BXO + 8] = _colform(p["bxo"])
    cols[:, :, C_BMO:C_BMO + 8] = _colform(p["bmo"])
    cols[:, :, C_BX1:C_BX1 + 32] = _colform(p["bx1"])
    cols[:, :, C_BM1:C_BM1 + 32] = _colform(p["bm1"])
    cols[:, :, C_BX2:C_BX2 + 8] = _colform(p["bx2"])
    cols[:, :, C_BM2:C_BM2 + 8] = _colform(p["bm2"])
    for base, gk, bk_ in ((C_LN1X, "ln1x_g", "ln1x_b"),
                          (C_LN1M, "ln1m_g", "ln1m_b"),
                          (C_LN2X, "ln2x_g", "ln2x_b"),
                          (C_LN2M, "ln2m_g", "ln2m_b"),
                          (C_LN3X, "ln3x_g", "ln3x_b"),
                          (C_LN3M, "ln3m_g", "ln3m_b")):
        cols[:, :, base:base + 8] = _colform(p[gk])
        cols[:, :, base + 8:base + 16] = _colform(p[bk_])
    rows_ = np.ascontiguousarray(
        np.stack([p["bk"], p["bv"]], axis=1).astype(np.float32))
    ionc = np.concatenate([np.eye(128, dtype=np.float32),
                           np.ones((128, 512), np.float32)], axis=1)
    base = {
        "Wq": p["Wq"], "Wk": p["Wk"], "Wv": p["Wv"],
        "Wxo": p["Wxo"], "Wmo": p["Wmo"],
        "Wx1": p["Wx1"], "Wx2": p["Wx2"],
        "Wm1": p["Wm1"], "Wm2": p["Wm2"],
        "cols": cols, "rows": rows_, "ionc": ionc,
    }
    return base


def run(src_x, src_m, src_mask, params, n_layers=L, **run_kwargs):
    nc = _get_nc(n_layers)
    p = {k: np.ascontiguousarray(np.asarray(v), dtype=np.float32)
         for k, v in params.items()}
    base = _pack_params(p, n_layers)
    src_x = np.asarray(src_x, dtype=np.float32)
    src_m = np.asarray(src_m, dtype=np.float32)
    in_maps = [dict(base, x0=np.ascontiguousarray(src_x[b]),
                    m0=np.ascontiguousarray(src_m[b]))
               for b in range(N_CORES)]
    res = run_bass_kernel_spmd(nc, in_maps, list(range(N_CORES)), **run_kwargs)
    sx = np.stack([res.results[b]["sx_out"] for b in range(N_CORES)])
    sm = np.stack([res.results[b]["sm_out"] for b in range(N_CORES)])
    attn = np.stack([res.results[b]["attn_out"] for b in range(N_CORES)],
                    axis=1)
    return (sx, sm, attn), res


def kernel(src_x, src_m, src_mask, params):
    (sx, sm, attn), _ = run(src_x, src_m, src_mask, params)
    return sx, sm, attn


# revision 13
# speedup vs baseline: 15.9588x; 15.9588x over previous
"""Trainium2 Bass/Tile kernel for a 6-layer dual-stream encoder.

Strategy: data-parallel over batch (B=8) across the 8 NeuronCores; each core
runs the full 6-layer encoder for one batch element. Activations are kept
feature-major (D on partitions, S on the free dim) so weights serve as matmul
lhsT in their natural layout. LayerNorm statistics are computed with
ones-vector matmuls (partition-dim reduction). Attention computes both
Q@K^T (softmax + attention-map output) and K@Q^T (transposed scores for the
attn@K / attn@V contractions). Q/K/V and the transposed exp-scores are held
in bf16 to fit SBUF; residual streams and everything else are fp32.
"""

import sys

if "/opt/trn_rl_repo" not in sys.path:
    sys.path.insert(0, "/opt/trn_rl_repo")

import numpy as np

import concourse.bass as bass
import concourse.bacc as bacc
import concourse.tile as tile
from concourse import mybir
from concourse.bass_utils import run_bass_kernel_spmd

F32 = mybir.dt.float32
F32R = mybir.dt.float32r
BF16 = mybir.dt.bfloat16
AF = mybir.ActivationFunctionType

L, D, H, DK, DF = 6, 1024, 16, 64, 4096
B, S = 8, 512
HD = H * DK            # 1024
KD = D // 128          # 8 feature tiles
SP = S // 128          # 4 token tiles
NPAIR = HD // 128      # 8 head pairs
NBLK = 4               # DF blocks of 1024
EPS = 1e-5
N_CORES = 8

# column-pack layout (within cols[l], shape (128, NCOL)):
C_BQ, C_BK, C_BXO, C_BMO = 0, 8, 16, 24
C_BX1, C_BM1 = 32, 64
C_BX2, C_BM2 = 96, 104
C_LN1X, C_LN1M = 112, 128      # g at +0, b at +8
C_LN2X, C_LN2M = 144, 160
C_LN3X, C_LN3M = 176, 192
NCOL = 208


def _build(n_layers=L, repeat=1):
    nc = bacc.Bacc("TRN2", target_bir_lowering=False, debug=False,
                   enable_asserts=False, num_devices=N_CORES)

    x0 = nc.dram_tensor("x0", [S, D], F32R, kind="ExternalInput").ap()
    m0 = nc.dram_tensor("m0", [S, D], F32R, kind="ExternalInput").ap()
    wq = nc.dram_tensor("Wq", [n_layers, D, HD], F32R, kind="ExternalInput").ap()
    wk = nc.dram_tensor("Wk", [n_layers, D, HD], F32R, kind="ExternalInput").ap()
    wv = nc.dram_tensor("Wv", [n_layers, D, HD], F32R, kind="ExternalInput").ap()
    wxo = nc.dram_tensor("Wxo", [n_layers, HD, D], F32R, kind="ExternalInput").ap()
    wmo = nc.dram_tensor("Wmo", [n_layers, HD, D], F32R, kind="ExternalInput").ap()
    wx1 = nc.dram_tensor("Wx1", [n_layers, D, DF], F32R, kind="ExternalInput").ap()
    wx2 = nc.dram_tensor("Wx2", [n_layers, DF, D], F32R, kind="ExternalInput").ap()
    wm1 = nc.dram_tensor("Wm1", [n_layers, D, DF], F32R, kind="ExternalInput").ap()
    wm2 = nc.dram_tensor("Wm2", [n_layers, DF, D], F32R, kind="ExternalInput").ap()
    cols = nc.dram_tensor("cols", [n_layers, 128, NCOL], F32, kind="ExternalInput").ap()
    rows = nc.dram_tensor("rows", [n_layers, 2, HD], F32, kind="ExternalInput").ap()
    ion = nc.dram_tensor("ionc", [128, 640], F32R, kind="ExternalInput").ap()

    sx_out = nc.dram_tensor("sx_out", [S, D], F32, kind="ExternalOutput").ap()
    sm_out = nc.dram_tensor("sm_out", [S, D], F32, kind="ExternalOutput").ap()
    attn_out = nc.dram_tensor("attn_out", [n_layers, H, S, S], F32,
                              kind="ExternalOutput").ap()

    with tile.TileContext(nc) as tc, \
            nc.allow_low_precision(reason="fp32r matmul operands"):
        with tc.tile_pool(name="sb", bufs=1) as sb, \
             tc.tile_pool(name="ps", bufs=8, space="PSUM") as psp:

            ion_t = sb.tile([128, 640], F32R, tag="ion")
            nc.sync.dma_start(out=ion_t, in_=ion)
            ident = ion_t[:, 0:128]
            ones_col = ion_t[:, 128:129]     # (128,1) fp32 ones
            ones_row = ion_t[0:1, 128:256]   # (1,128) fp32 ones
            onesb = sb.tile([128, 16], BF16, tag="ionb")
            nc.vector.memset(onesb, 1.0)
            ones_col_b = onesb[:, 0:1]       # (128,1) bf16 ones
            eps_t = sb.tile([128, 1], F32, tag="epsc")
            nc.vector.memset(eps_t, EPS)

            def mm(out, lhsT, rhs, **kw):
                if lhsT.dtype == F32:
                    lhsT = lhsT.bitcast(F32R)
                if rhs.dtype == F32:
                    rhs = rhs.bitcast(F32R)
                nc.tensor.matmul(out, lhsT, rhs, **kw)

            def mtr(out, in_, iden):
                nc.tensor.transpose(out.bitcast(F32R), in_.bitcast(F32R),
                                    iden.bitcast(F32R))

            def ln(src3, cbase, dst3):
                """LayerNorm over the partition (feature) dim.
                src3/dst3: [128, KD, S] fp32; cbase: colpack base (g at cbase,
                b at cbase+KD)."""
                ps1 = psp.tile([1, S], F32, tag="ps")
                ps2 = psp.tile([1, S], F32, tag="ps")
                for d in range(KD):
                    sq = sb.tile([128, S], F32R, tag="sq", bufs=3)
                    nc.scalar.activation(sq, src3[:, d, :], AF.Square)
                    mm(ps1, ones_col, src3[:, d, :],
                                     start=(d == 0), stop=(d == KD - 1))
                    mm(ps2, ones_col, sq,
                                     start=(d == 0), stop=(d == KD - 1))
                vr = sb.tile([1, 4, S], F32R, tag="vrow", bufs=1)
                mu, var, rstd, murstd = (vr[:, 0, :], vr[:, 1, :],
                                         vr[:, 2, :], vr[:, 3, :])
                nc.scalar.activation(mu, ps1, AF.Copy, scale=1.0 / D)
                nc.scalar.activation(var, ps2, AF.Copy, scale=1.0 / D)
                nc.vector.tensor_mul(rstd, mu, mu)           # mu^2 (scratch)
                nc.vector.tensor_sub(var, var, rstd)         # var
                nc.scalar.activation(var, var, AF.Sqrt,
                                     bias=eps_t[0:1, :])  # std
                nc.vector.reciprocal(rstd, var)              # 1/std
                nc.vector.tensor_mul(murstd, mu, rstd)       # mu/std
                psr = psp.tile([128, S], F32, tag="ps")
                psm = psp.tile([128, S], F32, tag="ps")
                mm(psr, ones_row, rstd, start=True, stop=True)
                mm(psm, ones_row, murstd, start=True, stop=True)
                for d in range(KD):
                    t = sb.tile([128, S], F32R, tag="sq", bufs=3)
                    nc.vector.tensor_mul(t, src3[:, d, :], psr)
                    nc.vector.tensor_sub(t, t, psm)
                    nc.vector.tensor_scalar(
                        dst3[:, d, :], t,
                        colp[:, cbase + d:cbase + d + 1],
                        colp[:, cbase + KD + d:cbase + KD + d + 1],
                        op0=mybir.AluOpType.mult, op1=mybir.AluOpType.add)

            def wquarter(wsrc, l, q, colsl=None):
                """Load a (256, N<=1024) row-quarter of a weight matrix as
                [128, 2, N] (contraction rows on partitions)."""
                if colsl is None:
                    src = wsrc[l][q * 256:(q + 1) * 256, :]
                else:
                    src = wsrc[l][q * 256:(q + 1) * 256, colsl[0]:colsl[1]]
                n = src.shape[1]
                t = sb.tile([128, 2, n], F32R, tag="wslab", bufs=3)
                nc.sync.dma_start(
                    out=t, in_=src.rearrange("(kd p) n -> p kd n", p=128))
                return t

            # ---------- input transposes: token-major -> feature-major ----
            sxv = sb.tile([128, KD, S], F32R, tag="s5")
            smv = sb.tile([128, KD, S], F32R, tag="s6")
            for (src, dst3) in ((x0, sxv), (m0, smv)):
                for i in range(SP):
                    stg = sb.tile([128, D], F32R, tag="attn_sb", bufs=2)
                    nc.sync.dma_start(out=stg, in_=src[i * 128:(i + 1) * 128, :])
                    for d in range(KD):
                        pt = psp.tile([128, 128], F32, tag="ps")
                        mtr(pt, stg[:, d * 128:(d + 1) * 128],
                                            ident)
                        nc.vector.tensor_copy(
                            dst3[:, d, i * 128:(i + 1) * 128], pt)

            import contextlib
            rep_cm = (tc.For_i(0, repeat, 1) if repeat > 1
                      else contextlib.nullcontext())
            with rep_cm:
              for l in range(n_layers):
                colp = sb.tile([128, NCOL], F32, tag="colp", bufs=2)
                nc.sync.dma_start(out=colp, in_=cols[l])
                bkbc = sb.tile([128, 2, HD], F32, tag="bkbc")
                for r in range(2):
                    rsrc = rows[l, r]
                    bsrc = bass.AP(tensor=rsrc.tensor, offset=rsrc.offset,
                                   ap=[[0, 128]] + list(rsrc.ap))
                    nc.gpsimd.dma_start(out=bkbc[:, r, :], in_=bsrc)

                # ---------- A: LayerNorm 1 on both streams ----------
                xt3 = sb.tile([128, KD, S], F32R, tag="s1")
                mt3 = sb.tile([128, KD, S], F32R, tag="s2")
                ln(sxv, C_LN1X, xt3)
                ln(smv, C_LN1M, mt3)

                # ---------- B: projections ----------
                qkt = sb.tile([128, NPAIR, 1024], BF16, tag="s3")
                kv = sb.tile([128, SP, 2048], BF16, tag="s4")

                # KT: feature-major K^T -> qkt[:, pair, 512:1024]
                pls = [psp.tile([128, S], F32, tag="ps", name=f"pls{_i}")
                       for _i in range(8)]
                for q in range(4):
                    wt = wquarter(wk, l, q)
                    for k2 in range(2):
                        kd = q * 2 + k2
                        for m in range(NPAIR):
                            mm(
                                pls[m], wt[:, k2, m * 128:(m + 1) * 128],
                                xt3[:, kd, :],
                                start=(kd == 0), stop=(kd == KD - 1))
                for m in range(NPAIR):
                    nc.scalar.activation(qkt[:, m, 512:1024], pls[m], AF.Identity,
                                         bias=colp[:, C_BK + m:C_BK + m + 1])

                # K: token-major -> kv[:, j, 0:1024]
                pls = [psp.tile([128, S], F32, tag="ps", name=f"pls{_i}")
                       for _i in range(8)]
                for q in range(4):
                    wt = wquarter(wk, l, q)
                    for k2 in range(2):
                        kd = q * 2 + k2
                        for j in range(SP):
                            for n in range(2):
                                mm(
                                    pls[j * 2 + n],
                                    xt3[:, kd, j * 128:(j + 1) * 128],
                                    wt[:, k2, n * 512:(n + 1) * 512],
                                    start=(kd == 0), stop=(kd == KD - 1))
                for j in range(SP):
                    for n in range(2):
                        nc.vector.tensor_add(
                            kv[:, j, n * 512:(n + 1) * 512], pls[j * 2 + n],
                            bkbc[:, 0, n * 512:(n + 1) * 512])

                # V: token-major -> kv[:, j, 1024:2048]
                pls = [psp.tile([128, S], F32, tag="ps", name=f"pls{_i}")
                       for _i in range(8)]
                for q in range(4):
                    wt = wquarter(wv, l, q)
                    for k2 in range(2):
                        kd = q * 2 + k2
                        for j in range(SP):
                            for n in range(2):
                                mm(
                                    pls[j * 2 + n],
                                    mt3[:, kd, j * 128:(j + 1) * 128],
                                    wt[:, k2, n * 512:(n + 1) * 512],
                                    start=(kd == 0), stop=(kd == KD - 1))
                for j in range(SP):
                    for n in range(2):
                        nc.vector.tensor_add(
                            kv[:, j, 1024 + n * 512:1024 + (n + 1) * 512],
                            pls[j * 2 + n],
                            bkbc[:, 1, n * 512:(n + 1) * 512])

                # QT: feature-major Q^T -> qkt[:, pair, 0:512]
                pls = [psp.tile([128, S], F32, tag="ps", name=f"pls{_i}")
                       for _i in range(8)]
                for q in range(4):
                    wt = wquarter(wq, l, q)
                    for k2 in range(2):
                        kd = q * 2 + k2
                        for m in range(NPAIR):
                            mm(
                                pls[m], wt[:, k2, m * 128:(m + 1) * 128],
                                xt3[:, kd, :],
                                start=(kd == 0), stop=(kd == KD - 1))
                for m in range(NPAIR):
                    nc.scalar.activation(qkt[:, m, 0:512], pls[m], AF.Identity,
                                         bias=colp[:, C_BQ + m:C_BQ + m + 1])

                # ---------- C: attention ----------
                kbt = sb.tile([128, NPAIR, S], F32R, tag="s7")
                vbt = sb.tile([128, NPAIR, S], F32R, tag="s8")
                for h in range(H):
                    hd, po = h // 2, (h % 2) * 64
                    att = sb.tile([128, SP, S], F32, tag="attn_sb", bufs=2)
                    rec = sb.tile([128, 520], F32, tag="rec", bufs=2)
                    for i in range(SP):
                        pd = psp.tile([128, S], F32, tag="ps")
                        mm(
                            pd, qkt[po:po + 64, hd, i * 128:(i + 1) * 128],
                            qkt[po:po + 64, hd, 512:1024],
                            start=True, stop=True)
                        nc.scalar.activation(
                            att[:, i, :], pd, AF.Exp, scale=0.125,
                            accum_out=rec[:, 512 + i:513 + i])
                    nc.vector.reciprocal(rec[:, 516:520], rec[:, 512:516])
                    for i in range(SP):
                        nc.vector.tensor_scalar_mul(
                            att[:, i, :], att[:, i, :],
                            rec[:, 516 + i:517 + i])
                    nc.sync.dma_start(
                        out=attn_out[l, h].rearrange("(i p) j -> p i j", p=128),
                        in_=att)
                    # transposed scores
                    ext = sb.tile([128, SP, S], BF16, tag="expT", bufs=1)
                    for j in range(SP):
                        pdt = psp.tile([128, S], F32, tag="ps")
                        mm(
                            pdt,
                            qkt[po:po + 64, hd, 512 + j * 128:512 + (j + 1) * 128],
                            qkt[po:po + 64, hd, 0:512],
                            start=True, stop=True)
                        nc.scalar.activation(ext[:, j, :], pdt, AF.Exp,
                                             scale=0.125)
                    pss = psp.tile([1, S], F32, tag="ps")
                    for j in range(SP):
                        mm(pss, ones_col_b, ext[:, j, :],
                                         start=(j == 0), stop=(j == SP - 1))
                    rrow = sb.tile([1, S], F32R, tag="rrow", bufs=1)
                    nc.vector.reciprocal(rrow, pss)
                    prb = psp.tile([128, S], F32, tag="ps")
                    mm(prb, ones_row, rrow, start=True, stop=True)
                    nc.vector.tensor_copy(rec[:, 0:512], prb)
                    pkb = psp.tile([64, S], F32, tag="ps")
                    pvb = psp.tile([64, S], F32, tag="ps")
                    for j in range(SP):
                        mm(pkb, kv[:, j, h * 64:(h + 1) * 64],
                                         ext[:, j, :],
                                         start=(j == 0), stop=(j == SP - 1))
                        mm(
                            pvb, kv[:, j, 1024 + h * 64:1024 + (h + 1) * 64],
                            ext[:, j, :],
                            start=(j == 0), stop=(j == SP - 1))
                    nc.vector.tensor_mul(kbt[po:po + 64, hd, :], pkb,
                                         rec[0:64, 0:512])
                    nc.vector.tensor_mul(vbt[po:po + 64, hd, :], pvb,
                                         rec[0:64, 0:512])

                # ---------- D: output projections + residual + LN2 --------
                xat = sb.tile([128, KD, S], F32R, tag="s3")
                mat = sb.tile([128, KD, S], F32R, tag="s4")
                for (wsrc, rhs3, dst3, cb) in ((wxo, kbt, xat, C_BXO),
                                               (wmo, vbt, mat, C_BMO)):
                    pls = [psp.tile([128, S], F32, tag="ps", name=f"pls{_i}")
                       for _i in range(8)]
                    for q in range(4):
                        wt = wquarter(wsrc, l, q)
                        for k2 in range(2):
                            kh = q * 2 + k2
                            for m in range(KD):
                                mm(
                                    pls[m], wt[:, k2, m * 128:(m + 1) * 128],
                                    rhs3[:, kh, :],
                                    start=(kh == 0), stop=(kh == NPAIR - 1))
                    for m in range(KD):
                        nc.scalar.activation(dst3[:, m, :], pls[m], AF.Identity,
                                             bias=colp[:, cb + m:cb + m + 1])

                s2px = sb.tile([128, KD, S], F32R, tag="s7")
                for d in range(KD):
                    nc.vector.tensor_add(s2px[:, d, :], sxv[:, d, :],
                                         xat[:, d, :])
                sx2 = sb.tile([128, KD, S], F32, tag="s1")
                ln(s2px, C_LN2X, sx2)
                s2pm = sb.tile([128, KD, S], F32R, tag="s8")
                for d in range(KD):
                    nc.vector.tensor_add(s2pm[:, d, :], smv[:, d, :],
                                         mat[:, d, :])
                sm2 = sb.tile([128, KD, S], F32, tag="s2")
                ln(s2pm, C_LN2M, sm2)

                # ---------- E: FFN on both streams ----------
                for (src3, w1, w2, cb1, cb2, res3, cln, dstv) in (
                        (xat, wx1, wx2, C_BX1, C_BX2, sx2, C_LN3X, sxv),
                        (mat, wm1, wm2, C_BM1, C_BM2, sm2, C_LN3M, smv)):
                    fxa = sb.tile([128, KD, S], F32R, tag="s8")
                    for blk in range(NBLK):
                        hxb = sb.tile([128, 8, S], F32R, tag="s7")
                        pls = [psp.tile([128, S], F32, tag="ps", name=f"plsf{_i}")
                               for _i in range(8)]
                        for q in range(4):
                            wt = wquarter(w1, l, q,
                                          colsl=(blk * 1024, (blk + 1) * 1024))
                            for k2 in range(2):
                                kd = q * 2 + k2
                                for df in range(8):
                                    mm(
                                        pls[df],
                                        wt[:, k2, df * 128:(df + 1) * 128],
                                        src3[:, kd, :],
                                        start=(kd == 0), stop=(kd == KD - 1))
                        for df in range(8):
                            c = cb1 + blk * 8 + df
                            nc.scalar.activation(hxb[:, df, :], pls[df],
                                                 AF.Relu,
                                                 bias=colp[:, c:c + 1])
                        pls2 = [psp.tile([128, S], F32, tag="ps", name=f"pls2{_i}")
                                for _i in range(8)]
                        for q2 in range(4):
                            src = w2[l][blk * 1024 + q2 * 256:
                                        blk * 1024 + (q2 + 1) * 256, :]
                            wt2 = sb.tile([128, 2, D], F32R, tag="wslab",
                                          bufs=3)
                            nc.sync.dma_start(
                                out=wt2,
                                in_=src.rearrange("(df p) n -> p df n", p=128))
                            for d2 in range(2):
                                df = q2 * 2 + d2
                                for m in range(KD):
                                    mm(
                                        pls2[m],
                                        wt2[:, d2, m * 128:(m + 1) * 128],
                                        hxb[:, df, :],
                                        start=(df == 0), stop=(df == 7))
                        for m in range(KD):
                            if blk == 0:
                                nc.scalar.activation(
                                    fxa[:, m, :], pls2[m], AF.Identity,
                                    bias=colp[:, cb2 + m:cb2 + m + 1])
                            else:
                                nc.vector.tensor_add(fxa[:, m, :],
                                                     fxa[:, m, :], pls2[m])
                    for m in range(KD):
                        nc.vector.tensor_add(fxa[:, m, :], fxa[:, m, :],
                                             res3[:, m, :])
                    ln(fxa, cln, dstv)

            # ---------- output transposes ----------
            for (src3, dst) in ((sxv, sx_out), (smv, sm_out)):
                for i in range(SP):
                    stg = sb.tile([128, D], F32, tag="attn_sb", bufs=2)
                    for d in range(KD):
                        pt = psp.tile([128, 128], F32, tag="ps")
                        mtr(
                            pt, src3[:, d, i * 128:(i + 1) * 128], ident)
                        nc.vector.tensor_copy(stg[:, d * 128:(d + 1) * 128], pt)
                    nc.sync.dma_start(out=dst[i * 128:(i + 1) * 128, :],
                                      in_=stg)

    nc.compile()
    return nc


_NC_CACHE = {}


def _get_nc(n_layers=L, repeat=1):
    key = (n_layers, repeat)
    if key not in _NC_CACHE:
        _NC_CACHE[key] = _build(n_layers, repeat)
    return _NC_CACHE[key]


def _colform(v):
    """(nl, n) per-feature vectors -> (nl, 128, n//128) column form."""
    nl, n = v.shape
    return np.ascontiguousarray(
        v.reshape(nl, n // 128, 128).transpose(0, 2, 1))


def _pack_params(p, n_layers):
    cols = np.zeros((n_layers, 128, NCOL), np.float32)
    cols[:, :, C_BQ:C_BQ + 8] = _colform(p["bq"])
    cols[:, :, C_BK:C_BK + 8] = _colform(p["bk"])
    cols[:, :, C_BXO:C_BXO + 8] = _colform(p["bxo"])
    cols[:, :, C_BMO:C_BMO + 8] = _colform(p["bmo"])
    cols[:, :, C_BX1:C_BX1 + 32] = _colform(p["bx1"])
    cols[:, :, C_BM1:C_BM1 + 32] = _colform(p["bm1"])
    cols[:, :, C_BX2:C_BX2 + 8] = _colform(p["bx2"])
    cols[:, :, C_BM2:C_BM2 + 8] = _colform(p["bm2"])
    for base, gk, bk_ in ((C_LN1X, "ln1x_g", "ln1x_b"),
                          (C_LN1M, "ln1m_g", "ln1m_b"),
                          (C_LN2X, "ln2x_g", "ln2x_b"),
                          (C_LN2M, "ln2m_g", "ln2m_b"),
                          (C_LN3X, "ln3x_g", "ln3x_b"),
                          (C_LN3M, "ln3m_g", "ln3m_b")):
        cols[:, :, base:base + 8] = _colform(p[gk])
        cols[:, :, base + 8:base + 16] = _colform(p[bk_])
    rows_ = np.ascontiguousarray(
        np.stack([p["bk"], p["bv"]], axis=1).astype(np.float32))
    ionc = np.concatenate([np.eye(128, dtype=np.float32),
                           np.ones((128, 512), np.float32)], axis=1)
    base = {
        "Wq": p["Wq"], "Wk": p["Wk"], "Wv": p["Wv"],
        "Wxo": p["Wxo"], "Wmo": p["Wmo"],
        "Wx1": p["Wx1"], "Wx2": p["Wx2"],
        "Wm1": p["Wm1"], "Wm2": p["Wm2"],
        "cols": cols, "rows": rows_, "ionc": ionc,
    }
    return base


def run(src_x, src_m, src_mask, params, n_layers=L, **run_kwargs):
    nc = _get_nc(n_layers)
    p = {k: np.ascontiguousarray(np.asarray(v), dtype=np.float32)
         for k, v in params.items()}
    base = _pack_params(p, n_layers)
    src_x = np.asarray(src_x, dtype=np.float32)
    src_m = np.asarray(src_m, dtype=np.float32)
    in_maps = [dict(base, x0=np.ascontiguousarray(src_x[b]),
                    m0=np.ascontiguousarray(src_m[b]))
               for b in range(N_CORES)]
    res = run_bass_kernel_spmd(nc, in_maps, list(range(N_CORES)), **run_kwargs)
    sx = np.stack([res.results[b]["sx_out"] for b in range(N_CORES)])
    sm = np.stack([res.results[b]["sm_out"] for b in range(N_CORES)])
    attn = np.stack([res.results[b]["attn_out"] for b in range(N_CORES)],
                    axis=1)
    return (sx, sm, attn), res


def kernel(src_x, src_m, src_mask, params):
    (sx, sm, attn), _ = run(src_x, src_m, src_mask, params)
    return sx, sm, attn
